# revision 11
# baseline (speedup 1.0000x reference)
"""Causal multi-head self-attention (B=4, T=2048, C=1024, H=16) on 8 TRN2 NeuronCores.

Sharding: core = b*2 + g  (b = batch 0..3, g = head-group 0..1 of 8 heads each).
Data parallel over batch; tensor parallel over heads (column-parallel W_attn,
row-parallel W_proj). Each core returns a partial (T, C) output; the host sums
the two partials per batch (the TP all-reduce happens in the unshard step).

Per-core device kernel (bf16 matmuls, f32 accumulation), per 512-wide q chunk:
  1. qT/kT projection with heads on partitions; head pairs share a 128-row tile
  2. v_aug projection in natural [t, c] layout with an all-ones column per head
     (the ones column turns the softmax denominator into row 64 of the y^T psum)
  3. attention in transposed [s, q] layout, one 128-key block per round:
       the head pair's two S^T matmuls go to disjoint PE row groups (0:64 /
       64:128) and disjoint psum banks of one [128,1024] tile, issued
       back-to-back so they stream through the array CONCURRENTLY
       E = exp(S/8) on ScalarE, one activation per round covering both heads
       causal mask = multiplicative 0/1 tile on diagonal blocks (DVE)
       y^T_aug accumulates v_aug^T @ E per head over rounds in PSUM
  4. projection/output-projection matmuls are chopped into ~2-matmul units and
     interleaved between rounds so the PE never idles while ScalarE runs exp
     (idle PE re-throttles the HAM clock gate to 1.2 GHz); output projections
     are deferred to the last chunk, whose attention has no other filler left.
"""

import numpy as np
import ml_dtypes

B, T, C, H = 4, 2048, 1024, 16
HS = C // H          # 64
NHL = 8              # local heads per core
KT = C // 128        # 8 contraction subtiles
NQC = T // 512       # 4 query chunks
NTB = T // 128       # 16 t-blocks
Bb16 = ml_dtypes.bfloat16

_CACHE = {}

# linear index of attention round (qc, hp, j); lookahead for filler deadlines
_OFF = {0: 0, 1: 16, 2: 48, 3: 96}


def _lin(qc, hp, j):
    return _OFF[qc] + hp * (4 * qc + 4) + j


def _build():
    import concourse.bass as bass
    import concourse.bacc as bacc
    import concourse.tile as tile
    import concourse.mybir as mybir

    BF = mybir.dt.bfloat16
    F32 = mybir.dt.float32
    AF = mybir.ActivationFunctionType

    # Calibrate the Tile scheduler's ScalarE cost model: the stock numbers
    # under-charge ACTIVATE by the ~350-cycle fixed overhead, so the static
    # per-engine instruction order places exp-dependent matmuls too early in
    # the PE FIFO, where they stall at the queue head while ready filler
    # matmuls sit behind them.  Overestimating is safe (late placement of an
    # already-ready instruction costs nothing).
    from concourse import hw_specs
    hw_specs.TRN2Spec.ACCESS_CYCLES[
        (bass.MemorySpace.SBUF, mybir.EngineType.Activation)] = 480

    nc = bacc.Bacc("TRN2", target_bir_lowering=False, debug=False, num_devices=8)
    xT = nc.dram_tensor("xT", [C, T], BF, kind="ExternalInput").ap()
    wqk = nc.dram_tensor("wqk", [C, 1024], BF, kind="ExternalInput").ap()
    wv = nc.dram_tensor("wv", [C, 520], BF, kind="ExternalInput").ap()
    wp = nc.dram_tensor("wp", [512, C], BF, kind="ExternalInput").ap()
    mask = nc.dram_tensor("mask", [128, 256], BF, kind="ExternalInput").ap()
    out = nc.dram_tensor("out", [T, C], BF, kind="ExternalOutput").ap()

    with tile.TileContext(nc) as tc:
        with tc.tile_pool(name="persist", bufs=1) as persist, \
             tc.tile_pool(name="mm", bufs=2, space="PSUM") as mmpool, \
             tc.tile_pool(name="s", bufs=2, space="PSUM") as spool, \
             tc.tile_pool(name="av", bufs=2, space="PSUM") as avpool, \
             tc.tile_pool(name="e", bufs=6) as epool, \
             tc.tile_pool(name="nrm", bufs=2) as nrmpool, \
             tc.tile_pool(name="bc", bufs=4) as bcpool, \
             tc.tile_pool(name="osb", bufs=3) as outpool:

            xT_sb = persist.tile([128, KT, T], BF, tag="xT")
            wqk_sb = persist.tile([128, KT, 1024], BF, tag="wqk")
            wv_sb = persist.tile([128, KT, 520], BF, tag="wv")
            wp_sb = persist.tile([128, 4, 1024], BF, tag="wp")
            mask_sb = persist.tile([128, 256], BF, tag="mask")
            qk_sb = persist.tile([128, 8, T], BF, tag="qk")
            v_sb = persist.tile([128, NTB, 520], BF, tag="v")
            yT_sb = persist.tile([128, 4, T], BF, tag="yT")
            ones_sb = persist.tile([1, 64], BF, tag="ones")
            nc.vector.memset(ones_sb[:], 1.0)

            # load order: exactly what the first projection chunk needs, first
            nc.sync.dma_start(wqk_sb[:, 0, 0:128], wqk[0:128, 0:128])
            nc.sync.dma_start(xT_sb[:, 0, 0:512], xT[0:128, 0:512])
            for m in range(1, 8):
                nc.sync.dma_start(wqk_sb[:, 0, m * 128:(m + 1) * 128],
                                  wqk[0:128, m * 128:(m + 1) * 128])
            for k in range(1, KT):
                nc.sync.dma_start(wqk_sb[:, k, :], wqk[k * 128:(k + 1) * 128, :])
                nc.sync.dma_start(xT_sb[:, k, 0:512], xT[k * 128:(k + 1) * 128, 0:512])
            for k in range(KT):
                nc.sync.dma_start(wv_sb[:, k, :], wv[k * 128:(k + 1) * 128, :])
            nc.sync.dma_start(mask_sb[:], mask[:])
            for k in range(KT):
                nc.sync.dma_start(xT_sb[:, k, 512:1024],
                                  xT[k * 128:(k + 1) * 128, 512:1024])
            for k in range(4):
                nc.sync.dma_start(wp_sb[:, k, :], wp[k * 128:(k + 1) * 128, :])
            for k in range(KT):
                nc.sync.dma_start(xT_sb[:, k, 1024:2048],
                                  xT[k * 128:(k + 1) * 128, 1024:2048])

            # ---------------- filler units -------------------------------
            # Each unit is ~2 matmuls of PE work; groups share a psum tile
            # via a closure holder.  `due` (linear round index) marks the
            # round BEFORE which the unit must be emitted (data prereq).

            units = []   # list of dicts {due, cost, fn}

            def qk_units(qcp, m, due):
                holder = {}
                def mk(k0):
                    def f(ctx="fill"):
                        if k0 == 0:
                            holder["ps"] = mmpool.tile([128, 512], F32, tag="mm", name=f"qk{qcp}_{m}")
                        ps = holder["ps"][:]
                        for k in (k0, k0 + 1):
                            nc.tensor.matmul(
                                ps, wqk_sb[:, k, m * 128:(m + 1) * 128],
                                xT_sb[:, k, qcp * 512:qcp * 512 + 512],
                                start=(k == 0), stop=(k == KT - 1))
                        if k0 == 6:
                            nc.vector.tensor_copy(
                                qk_sb[:, m, qcp * 512:qcp * 512 + 512], ps)
                    return f
                return [dict(due=due, cost=450, fn=mk(k0)) for k0 in (0, 2, 4, 6)]

            def v_units(j, due):
                holder = {}
                def big(k0):
                    def f(ctx="fill"):
                        if k0 == 0:
                            holder["ps"] = mmpool.tile([128, 512], F32, tag="mm", name=f"vp{j}")
                        ps = holder["ps"][:]
                        for k in (k0, k0 + 1):
                            nc.tensor.matmul(
                                ps, xT_sb[:, k, j * 128:(j + 1) * 128],
                                wv_sb[:, k, 0:512],
                                start=(k == 0), stop=(k == KT - 1))
                        if k0 == 6:
                            nc.vector.tensor_copy(v_sb[:, j, 0:512], ps)
                    return f
                def small(ctx="fill"):
                    ps2 = mmpool.tile([128, 8], F32, tag="mm", name=f"vp2_{j}")
                    for k in range(KT):
                        nc.tensor.matmul(
                            ps2[:], xT_sb[:, k, j * 128:(j + 1) * 128],
                            wv_sb[:, k, 512:520],
                            start=(k == 0), stop=(k == KT - 1))
                    nc.vector.tensor_copy(v_sb[:, j, 512:520], ps2[:])
                    vones = v_sb[:, j, :].rearrange("p (h e) -> p h e", e=65)[:, :, 64]
                    nc.vector.memset(vones, 1.0)
                us = [dict(due=due, cost=450, fn=big(k0)) for k0 in (0, 2, 4, 6)]
                us.append(dict(due=due, cost=300, fn=small))
                return us

            def outproj_units(qcv, tt):
                holder = {}
                t0 = qcv * 512 + tt * 128
                def half(n):
                    def f(ctx="fill"):
                        if n == 0:
                            holder["osb"] = outpool.tile([128, 1024], BF, tag="osb", name=f"osb{qcv}_{tt}")
                            if ctx == "tail" and (tt % 2 == 0):
                                # attention pools are free at the kernel tail:
                                # borrow an S psum tile (2 banks) so several
                                # output-projection groups can be in flight
                                holder["st"] = spool.tile(
                                    [128, 1024], F32, tag="s",
                                    name=f"ost{qcv}_{tt}")
                        if "st" in holder:
                            ops = holder["st"][:, n * 512:(n + 1) * 512]
                        else:
                            ops = mmpool.tile([128, 512], F32, tag="mm",
                                              name=f"op{qcv}_{tt}_{n}")[:]
                        for cp in range(4):
                            nc.tensor.matmul(
                                ops, yT_sb[:, cp, t0:t0 + 128],
                                wp_sb[:, cp, n * 512:(n + 1) * 512],
                                start=(cp == 0), stop=(cp == 3))
                        with nc.allow_low_precision(reason="bf16 partial output"):
                            if ctx == "tail":
                                nc.scalar.copy(
                                    holder["osb"][:, n * 512:(n + 1) * 512], ops)
                            else:
                                nc.vector.tensor_copy(
                                    holder["osb"][:, n * 512:(n + 1) * 512], ops)
                        if n == 1:
                            nc.sync.dma_start(out[t0:t0 + 128, :], holder["osb"][:])
                    return f
                return [dict(due=None, cost=900, fn=half(0)),
                        dict(due=None, cost=950, fn=half(1))]

            # proj work for chunks 1..3 (chunk 0 is done by the boot) with
            # per-round deadlines, 3 rounds of lookahead for exec latency
            LOOK = 3
            ent = []
            for qcp in range(1, NQC):
                for m in range(4):           # q of pair m: before pair m round 0
                    ent.append((_lin(qcp, m, 0), qk_units(qcp, m, None)))
                for hp in range(4):          # k of pair hp: before its block 4qcp
                    ent.append((_lin(qcp, hp, 4 * qcp), qk_units(qcp, 4 + hp, None)))
                for j in range(4 * qcp, 4 * qcp + 4):   # v(j): before round j
                    ent.append((_lin(qcp, 0, j), v_units(j, None)))
            for j in (2, 3):                 # chunk-0 v tail during early rounds
                ent.append((_lin(0, 0, j - 2), v_units(j, None)))
            ent.sort(key=lambda e: e[0])
            for due, us in ent:
                for u in us:
                    u["due"] = due
                    units.append(u)

            debt_box = [0.0]

            def pop_due(cur):
                for u in [u for u in units
                          if u["due"] is not None and u["due"] <= cur + LOOK]:
                    units.remove(u)
                    u["fn"]()
                    debt_box[0] -= u["cost"]

            # ---------------- attention ---------------------------------

            def emit_round(qc, hp, j, avA, avB):
                q0 = qc * 512
                t = j - 4 * qc
                w = 512 if t <= 0 else 512 - 128 * t
                qo = 512 - w
                sp = spool.tile([128, 1024], F32, tag="s", name=f"s{qc}_{hp}_{j}")
                for pb, off in ((0, 0), (64, 512)):
                    nc.tensor.matmul(
                        sp[:, off:off + w],
                        qk_sb[pb:pb + 64, 4 + hp, j * 128:(j + 1) * 128],
                        qk_sb[pb:pb + 64, hp, q0 + qo:q0 + 512],
                        start=True, stop=True, tile_position=(pb, 0))
                ep = epool.tile([128, 1024], BF, tag="e", name=f"e{qc}_{hp}_{j}")
                if t <= 0:
                    nc.scalar.activation(ep[:, 0:1024], sp[:, 0:1024],
                                         AF.Exp, scale=0.125)
                else:
                    sp3 = sp[:].rearrange("p (b c) -> p b c", b=2)[:, :, 0:w]
                    ep3 = ep[:].rearrange("p (b c) -> p b c", b=2)[:, :, 0:w]
                    nc.scalar.activation(ep3, sp3, AF.Exp, scale=0.125)
                if t >= 0:
                    # mask the triangular first 128 columns of each half
                    epm = ep[:].rearrange("p (b c) -> p b c", b=2)[:, :, 0:128]
                    m3 = mask_sb[:].rearrange("p (b c) -> p b c", b=2)
                    nc.vector.tensor_mul(epm, epm, m3)
                for h, av, off in ((2 * hp, avA, 0), (2 * hp + 1, avB, 512)):
                    nc.tensor.matmul(
                        av[:, qo:512], v_sb[:, j, h * 65:h * 65 + 65],
                        ep[:, off:off + w],
                        start=(j == 0), stop=(j == 4 * qc + 3))

            def make_norm(qc, half, yraw_sb, den8_sb):
                def f():
                    q0 = qc * 512
                    tail = (qc == 3 and half == 1)
                    r4 = nrmpool.tile([128, 512], F32, tag="r4",
                                      name=f"r4_{qc}_{half}")
                    nc.vector.reciprocal_approx_fast(r4[:], den8_sb[:, half, :])
                    for h in range(4 * half, 4 * half + 4):
                        pb = (h % 2) * 64
                        p32 = (h % 4) * 32
                        r1 = nrmpool.tile([1, 512], BF, tag="r1", name=f"r1_{qc}_{h}")
                        with nc.allow_low_precision(reason="softmax denom bf16"):
                            nc.vector.tensor_copy(r1[:], r4[p32:p32 + 1, :])
                        if tail:
                            bc = mmpool.tile([128, 512], F32, tag="mm",
                                             name=f"bc{qc}_{h}")[0:64, :]
                            nc.tensor.matmul(bc, ones_sb[:], r1[:],
                                             start=True, stop=True)
                        else:
                            bc = bcpool.tile([64, 512], BF, tag="bc",
                                             name=f"bc{qc}_{h}")[:]
                            nc.sync.dma_start(
                                bc, r1[:].unsqueeze(1).broadcast_to([1, 64, 512]))
                        with nc.allow_low_precision(reason="attention y bf16"):
                            nc.vector.tensor_mul(
                                yT_sb[pb:pb + 64, h // 2, q0:q0 + 512],
                                yraw_sb[:, h, :], bc)
                    if half == 1 and qc < 3:
                        for tt in range(4):
                            units.extend(outproj_units(qc, tt))
                return f

            def boot_qk_proj():
                # chunk-0 qT/kT projection with k as the OUTER loop: all 8
                # column-slot accumulators live at once (attention pools are
                # idle at kernel start, so all 8 PSUM banks are free) and the
                # first matmuls issue after only the first k-slice of DMA
                ps = []
                for m in range(8):
                    if m < 2:
                        ps.append(mmpool.tile([128, 512], F32, tag="mm",
                                              name=f"boot{m}"))
                    elif m < 6:
                        if m % 2 == 0:
                            st = spool.tile([128, 1024], F32, tag="s",
                                            name=f"boot{m}")
                        ps.append(st[:, (m % 2) * 512:(m % 2) * 512 + 512])
                    else:
                        ps.append(avpool.tile([128, 512], F32, tag="av",
                                              name=f"boot{m}"))
                for k in range(KT):
                    for m in range(8):
                        nc.tensor.matmul(
                            ps[m], wqk_sb[:, k, m * 128:(m + 1) * 128],
                            xT_sb[:, k, 0:512],
                            start=(k == 0), stop=(k == KT - 1))
                for m in range(8):
                    nc.vector.tensor_copy(qk_sb[:, m, 0:512], ps[m])

            boot_qk_proj()
            for us in (v_units(0, None), v_units(1, None)):
                for u in us:
                    u["fn"]()

            pending = []
            EST_ACT = {-1: 1147, 0: 1147, 1: 933, 2: 720, 3: 507}
            EST_MM = {-1: 790, 0: 810, 1: 620, 2: 440, 3: 260}
            for qc in range(NQC):
                den8_sb = nrmpool.tile([128, 2, 512], F32, tag="den8",
                                       name=f"den8{qc}")
                nc.vector.memset(den8_sb[:], 1.0)
                yraw_sb = nrmpool.tile([64, NHL, 512], BF, tag="yraw",
                                       name=f"yraw{qc}")
                for hp in range(4):
                    avA = avpool.tile([65, 512], F32, tag="av",
                                      name=f"avA_{qc}_{hp}")
                    avB = avpool.tile([65, 512], F32, tag="av",
                                      name=f"avB_{qc}_{hp}")
                    for j in range(4 * qc + 4):
                        cur = _lin(qc, hp, j)
                        pop_due(cur)
                        emit_round(qc, hp, j, avA, avB)
                        t = min(max(j - 4 * qc, -1), 3)
                        debt_box[0] += EST_ACT[t] - EST_MM[t]
                        while units and debt_box[0] > 200:
                            u = units.pop(0)
                            u["fn"]()
                            debt_box[0] -= u["cost"]
                        for trig, fn in pending[:]:
                            if cur >= trig:
                                pending.remove((trig, fn))
                                fn()
                                debt_box[0] -= 1500
                    for h, av in ((2 * hp, avA), (2 * hp + 1, avB)):
                        with nc.allow_low_precision(reason="attention y bf16"):
                            nc.vector.tensor_copy(yraw_sb[:, h, :], av[0:64, :])
                        p32 = (h % 4) * 32
                        nc.vector.tensor_copy(
                            den8_sb[p32:p32 + 1, h // 4, :], av[64:65, :])
                    if hp == 1:
                        pending.append((_lin(qc, 2, 1),
                                        make_norm(qc, 0, yraw_sb, den8_sb)))
                if qc < 3:
                    pending.append((_lin(qc + 1, 0, 1),
                                    make_norm(qc, 1, yraw_sb, den8_sb)))
                else:
                    # tail: drain leftover fillers first so the PE works while
                    # the last normalization chain runs on DVE
                    while units:
                        units.pop(0)["fn"]("tail")
                    make_norm(3, 1, yraw_sb, den8_sb)()
                    for tt in range(4):
                        for u in outproj_units(3, tt):
                            u["fn"]("tail")
            while units:
                units.pop(0)["fn"]("tail")
    nc.compile()
    return nc


def _get_nc():
    if "nc" not in _CACHE:
        _CACHE["nc"] = _build()
    return _CACHE["nc"]


def _host_prep(x, W_attn, W_proj):
    """Shard + lay out per-core inputs. Returns list of 8 in_maps."""
    x = np.asarray(x, dtype=np.float32)
    W_attn = np.asarray(W_attn, dtype=np.float32)
    W_proj = np.asarray(W_proj, dtype=np.float32)

    # triangular 128x128 mask, duplicated for the two heads of a pair; only
    # the first 128 columns of a diagonal key-block are ever partially masked
    s_idx = np.arange(128)[:, None]
    q_idx = np.arange(128)[None, :]
    tri = (s_idx <= q_idx).astype(np.float32)
    mask = np.ascontiguousarray(np.concatenate([tri, tri], axis=1)).astype(Bb16)

    xT_b = [np.ascontiguousarray(x[b].T).astype(Bb16) for b in range(B)]
    in_maps = []
    for core in range(8):
        b, g = core // 2, core % 2
        c0 = g * 512
        wqk_g = np.concatenate(
            [W_attn[:, c0:c0 + 512], W_attn[:, C + c0:C + c0 + 512]], axis=1
        ).astype(Bb16)
        vbase = W_attn[:, 2 * C + c0:2 * C + c0 + 512]
        wv_g = np.zeros((C, 520), dtype=np.float32)
        for h in range(NHL):
            wv_g[:, h * 65:h * 65 + 64] = vbase[:, h * 64:(h + 1) * 64]
        wp_g = np.ascontiguousarray(W_proj[c0:c0 + 512, :]).astype(Bb16)
        in_maps.append({
            "xT": xT_b[b],
            "wqk": np.ascontiguousarray(wqk_g),
            "wv": wv_g.astype(Bb16),
            "wp": wp_g,
            "mask": mask,
        })
    return in_maps


def kernel(x, W_attn, W_proj):
    from concourse import bass_utils

    nc = _get_nc()
    in_maps = _host_prep(x, W_attn, W_proj)
    res = bass_utils.run_bass_kernel_spmd(nc, in_maps, core_ids=list(range(8)))
    outs = [res.results[c]["out"] for c in range(8)]
    full = np.empty((B, T, C), dtype=np.float32)
    for b in range(B):
        full[b] = outs[2 * b].astype(np.float32) + outs[2 * b + 1].astype(np.float32)
    return full
